# revision 21
# baseline (speedup 1.0000x reference)
"""Trainium2 Bass kernel for nn_BiDGNBlock (moe_routing).

Strategy: data-parallel over batch across 8 NeuronCores (no collectives).
Each core computes one batch element end-to-end.

Key optimizations vs the 70.4us baseline:
  - Expert table We streamed as fp8e4 (x128 scale; the final layer_norm is
    scale-invariant so the scale never needs to be divided out) -- halves
    the dominant 8.4MB DMA stream. Activations stay fp16: measured rel err
    1.56e-2 < 2e-2 gate, with exact-fp32 routing (picks verified identical).
  - Router norm chain deleted: top-k indices are invariant to the positive
    per-row scale 1/||xp||, and the top-2 softmax gate is exactly 1.0.
  - Softmax without row-max (|energy/16| < 0.75), exp fused with the 1/16
    scale in one scalar-engine activation.
  - LayerNorms restructured: proj bias preloaded into PSUM, bn_stats reads
    PSUM directly, normalize runs on the scalar engine (Identity with
    per-row scale/bias), beta+residual precombined host-side.
  - Expert masks: first 16 experts' masks built inline on DVE; the other 48
    replicated across partitions via a 2-descriptor DRAM round-trip on the
    otherwise-empty Activation-engine HWDGE queue (the baseline's 8 serial
    SWDGE descriptors + scheduler head-of-line block cost ~6us).
  - PE kept warm (HAM k=8/8) with dummy matmuls through the attention
    phase and a burst before the expert phase (HAM drops to half clock
    after ~2us of low PE duty; the baseline ran experts at k=4 for 13us).
  - Single output DMA on the empty Activation queue (baseline's outputs
    queued behind the We stream on the sync queue).
"""

import sys
import numpy as np

sys.path.insert(0, "/opt/trn_rl_repo")

N_CORES = 8
B, C, T = 8, 64, 256
EXP = 32
KT = T // 128  # 2 k-tiles over the feature dim
WE_SCALE = 128.0

_CACHE: dict = {}

# fp32 blob layouts: (name, partitions, shape). cols = prod(shape[1:]).
BLOB_A_SPEC = [
    ("xtl", 128, (128, KT, C)), ("xtr", 128, (128, KT, C)),
    ("wqt", 128, (128, KT, T)), ("wkt", 128, (128, KT, T)),
    ("bqp", 128, (128, KT)), ("bkp", 128, (128, KT)),
]
BLOB_B1_SPEC = [
    ("wvt", 128, (128, KT, T)), ("wpt", 128, (128, KT, T)),
    ("wrt", 128, (128, 2 * KT, EXP)),
    ("ident", 64, (64, 64)), ("sel", 2, (2, 2, 128)),
    ("xlb", 64, (64, T)), ("xrb", 64, (64, T)),
    ("bvr", 64, (64, T)), ("bpr", 64, (64, T)),
    ("aglr", 64, (64, T)), ("agrr", 64, (64, T)),
]
BLOB_B2_SPEC = [
    ("e8", 128, (128, C)),
    ("mglr", 64, (64, T)), ("mgrr", 64, (64, T)),
    ("oblb", 64, (64, T)), ("obrb", 64, (64, T)),
    ("behs", 64, (64, T)),
    ("brp", 32, (32, 1)), ("cent", 32, (32, C)), ("eiota", 64, (64, 1)),
]


def _layout(spec):
    off = {}
    n = 0
    for name, parts, shape in spec:
        cols = int(np.prod(shape[1:]))
        off[name] = (n, parts, shape)
        n += cols
    return off, n


OFF_A, NA = _layout(BLOB_A_SPEC)
OFF_B1, NB1 = _layout(BLOB_B1_SPEC)
OFF_B2, NB2 = _layout(BLOB_B2_SPEC)
BLOB_OFF = {**OFF_A, **OFF_B1, **OFF_B2}


def _build():
    import concourse.bass as bass
    import concourse.mybir as mybir
    import concourse.tile as tile
    from concourse import bacc
    from contextlib import ExitStack

    dt = mybir.dt
    f32, f16, f8 = dt.float32, dt.float16, dt.float8e4
    AF = mybir.ActivationFunctionType
    OP = mybir.AluOpType

    nc = bacc.Bacc("TRN2", target_bir_lowering=False, debug=False,
                   num_devices=N_CORES)

    blobA_d = nc.dram_tensor("blobA", [128, NA], f32, kind="ExternalInput")
    blobB1_d = nc.dram_tensor("blobB1", [128, NB1], f32, kind="ExternalInput")
    blobB2_d = nc.dram_tensor("blobB2", [128, NB2], f32, kind="ExternalInput")
    weq_d = nc.dram_tensor("weq", [128, C, KT, T], f8, kind="ExternalInput")
    ol2_d = nc.dram_tensor("ol2", [C, T], f32, kind="ExternalOutput")
    or2_d = nc.dram_tensor("or2", [C, T], f32, kind="ExternalOutput")

    with tile.TileContext(nc) as tc, ExitStack() as ctx:
        cst = ctx.enter_context(tc.tile_pool(name="cst", bufs=1))
        wk = ctx.enter_context(tc.tile_pool(name="wk", bufs=2))
        sm = ctx.enter_context(tc.tile_pool(name="sm", bufs=2))
        msk_p = ctx.enter_context(tc.tile_pool(name="msk", bufs=2))
        asc_p = ctx.enter_context(tc.tile_pool(name="asc", bufs=3))
        ps = ctx.enter_context(tc.tile_pool(name="ps", bufs=2, space="PSUM"))
        proj_p = ctx.enter_context(tc.tile_pool(name="proj", bufs=2, space="PSUM"))
        moe_p = ctx.enter_context(tc.tile_pool(name="moe", bufs=1, space="PSUM"))
        warm_p = ctx.enter_context(tc.tile_pool(name="warm", bufs=1, space="PSUM"))
        dram = ctx.enter_context(tc.tile_pool(name="dram", bufs=1, space="DRAM"))

        # ---- input DMAs: sync (SP) HWDGE queue, FIFO priority order ----
        blobA = cst.tile([128, NA], f32, tag="blobA")
        nc.sync.dma_start(out=blobA, in_=blobA_d.ap())
        blobB1 = cst.tile([128, NB1], f32, tag="blobB1")
        nc.sync.dma_start(out=blobB1, in_=blobB1_d.ap())
        blobB2 = cst.tile([128, NB2], f32, tag="blobB2")
        nc.sync.dma_start(out=blobB2, in_=blobB2_d.ap())
        weq = cst.tile([128, C, KT, T], f8, tag="weq")
        wea = weq_d.ap()
        for ch in range(4):
            nc.sync.dma_start(out=weq[:, ch * 16:(ch + 1) * 16],
                              in_=wea[:, ch * 16:(ch + 1) * 16])

        def bview(blob, name):
            off, parts, shape = BLOB_OFF[name]
            cols = int(np.prod(shape[1:]))
            v = blob[0:parts, off:off + cols]
            if len(shape) == 3:
                v = v.rearrange("p (a b) -> p a b", a=shape[1])
            return v

        xtl = bview(blobA, "xtl")
        xtr = bview(blobA, "xtr")
        wqt = bview(blobA, "wqt")
        wkt = bview(blobA, "wkt")
        bqp = bview(blobA, "bqp")
        bkp = bview(blobA, "bkp")
        wvt = bview(blobB1, "wvt")
        wpt = bview(blobB1, "wpt")
        wrt = bview(blobB1, "wrt")
        ident = bview(blobB1, "ident")
        sel = bview(blobB1, "sel")
        xlb = bview(blobB1, "xlb")
        xrb = bview(blobB1, "xrb")
        bvr = bview(blobB1, "bvr")
        bpr = bview(blobB1, "bpr")
        aglr = bview(blobB1, "aglr")
        agrr = bview(blobB1, "agrr")
        e8 = bview(blobB2, "e8")
        mglr = bview(blobB2, "mglr")
        mgrr = bview(blobB2, "mgrr")
        oblb = bview(blobB2, "oblb")
        obrb = bview(blobB2, "obrb")
        behs = bview(blobB2, "behs")
        brp = bview(blobB2, "brp")
        cent = bview(blobB2, "cent")
        eiota = bview(blobB2, "eiota")

        # ---- PE warm-up from memset tiles + ACT table preloads ----
        wsrc = cst.tile([128, 512], f16, tag="wsrc")
        nc.vector.memset(wsrc, 0.5)
        pw = warm_p.tile([128, 512], f32, tag="warm")
        for wi in range(6):
            nc.tensor.matmul(pw, wsrc[:, 0:128], wsrc,
                             start=True, stop=True, skip_group_check=True)
        wact = cst.tile([1, 32], f32, tag="wact")
        nc.vector.memset(wact, 1.0)
        nc.scalar.activation(out=wact, in_=wact, func=AF.Exp)
        nc.scalar.activation(out=wact, in_=wact, func=AF.Sqrt)
        nc.scalar.activation(out=wact, in_=wact, func=AF.Identity)

        def dmy(dep, n=1, cols=96):
            # keep-warm matmul pinned in time by a data dependency: the
            # scheduler cannot hoist it ahead of `dep` (SBUF tile).
            kp = dep.shape[0]
            lhs = dep if dep.shape[-1] <= 64 else dep[:, 0:64]
            mov = wsrc if dep.dtype == f16 else wsrc32
            for _ in range(n):
                nc.tensor.matmul(pw[0:64, 0:cols],
                                 lhs, mov[0:kp, 0:cols],
                                 start=True, stop=True, skip_group_check=True)

        wsrc32 = cst.tile([128, 256], f32, tag="wsrc32")
        nc.vector.memset(wsrc32, 0.25)

        # ---- off-critical-path casts on gpsimd (idle early) ----
        e8h = cst.tile([128, C], f16, tag="e8h")
        nc.gpsimd.tensor_copy(e8h, e8)
        behh = cst.tile([C, T], f16, tag="behh")
        nc.gpsimd.tensor_copy(behh, behs)

        # ---- attention: q.T, k.T ----
        qt = wk.tile([128, KT, C], f32, tag="qt")
        ktl = wk.tile([128, KT, C], f32, tag="ktl")
        for (src, w, bias, dst) in [(xtl, wqt, bqp, qt), (xtr, wkt, bkp, ktl)]:
            for ut in range(KT):
                p = ps.tile([128, C], f32, tag="ps")
                for kt in range(KT):
                    nc.tensor.matmul(p, w[:, kt, ut * 128:(ut + 1) * 128],
                                     src[:, kt], start=(kt == 0), stop=(kt == KT - 1))
                nc.vector.tensor_scalar(out=dst[:, ut], in0=p,
                                        scalar1=bias[:, ut:ut + 1], scalar2=None,
                                        op0=OP.add)

        # ---- v = (x_l - x_r) @ Wv.T + bv  (natural layout [c, u]) ----
        xdt = wk.tile([128, KT, C], f32, tag="xdt")
        nc.vector.tensor_sub(xdt, xtl, xtr)
        pv = ps.tile([C, T], f32, tag="ps")
        for kt in range(KT):
            nc.tensor.matmul(pv, xdt[:, kt], wvt[:, kt],
                             start=(kt == 0), stop=(kt == KT - 1))
        v_sb = wk.tile([C, T], f32, tag="v")
        nc.vector.tensor_tensor(out=v_sb, in0=pv, in1=bvr, op=OP.add)

        # ---- energy -> exp(energy/16) directly (|arg| < 0.75, no row-max) ----
        pe_ = ps.tile([C, C], f32, tag="ps")
        for ut in range(KT):
            nc.tensor.matmul(pe_, qt[:, ut], ktl[:, ut],
                             start=(ut == 0), stop=(ut == KT - 1))
        attn = wk.tile([C, C], f32, tag="attn")
        nc.scalar.activation(out=attn, in_=pe_, func=AF.Exp, scale=1.0 / 16.0)
        rowsum = sm.tile([C, 1], f32, tag="rowsum")
        nc.vector.tensor_reduce(rowsum, attn, axis=mybir.AxisListType.X, op=OP.add)
        nc.vector.reciprocal(rowsum, rowsum)
        nc.vector.tensor_scalar_mul(attn, attn, rowsum)
        dmy(qt[:, 0], 4)

        # ---- attn.T ----
        pat = ps.tile([C, C], f32, tag="ps")
        nc.tensor.transpose(pat, attn, ident)
        attnT = wk.tile([C, C], f32, tag="attnT")
        nc.vector.tensor_copy(attnT, pat)

        # ---- proj psum bias pre-init (off critical path, DVE idle here) ----
        ppl = proj_p.tile([C, T], f32, tag="projl")
        ppr = proj_p.tile([C, T], f32, tag="projr")
        nc.vector.tensor_copy(ppl, bpr)
        nc.vector.tensor_copy(ppr, bpr)

        # ---- out_l.T / out_r.T  [u, c] ----
        oLT = wk.tile([128, KT, C], f32, tag="oLT")
        oRT = wk.tile([128, KT, C], f32, tag="oRT")
        for ut in range(KT):
            pl = ps.tile([128, C], f32, tag="ps")
            nc.tensor.matmul(pl, v_sb[:, ut * 128:(ut + 1) * 128], attnT,
                             start=True, stop=True)
            nc.vector.tensor_copy(oLT[:, ut], pl)
            pr = ps.tile([128, C], f32, tag="ps")
            nc.tensor.matmul(pr, v_sb[:, ut * 128:(ut + 1) * 128], attn,
                             start=True, stop=True)
            nc.scalar.copy(oRT[:, ut], pr)
        dmy(attn, 2)

        # ---- proj (accumulates onto pre-loaded bias) ----
        for ut in range(KT):
            nc.tensor.matmul(ppl, oLT[:, ut], wpt[:, ut],
                             start=False, stop=(ut == KT - 1))
        dmy(attnT, 1)
        for ut in range(KT):
            nc.tensor.matmul(ppr, oRT[:, ut], wpt[:, ut],
                             start=False, stop=(ut == KT - 1))
        dmy(attnT, 8)

        # ---- LayerNorm, engine-hop-minimized:
        #  DVE: stats, aggr, var+eps, 1/(var+eps), -mu   (one engine, no hops)
        #  SCALAR: rstd=sqrt(vinv), nmr=-mu*rstd, normalize=(x*rstd+nmr)
        #  DVE (or POOL for the second side): *gamma, +beta+resid ----
        def ln_start(src_ps, stag):
            stats = sm.tile([C, 6], f32, tag="st" + stag)
            nc.vector.bn_stats(out=stats, in_=src_ps)
            mv = sm.tile([C, 2], f32, tag="mv" + stag)
            nc.vector.bn_aggr(out=mv, in_=stats)
            vinv = sm.tile([C, 1], f32, tag="vi" + stag)
            nc.vector.tensor_scalar(out=vinv, in0=mv[:, 1:2], scalar1=1e-5,
                                    scalar2=None, op0=OP.add)
            nc.vector.reciprocal(vinv, vinv)
            nmu = sm.tile([C, 1], f32, tag="nu" + stag)
            nc.vector.tensor_scalar(out=nmu, in0=mv[:, 0:1], scalar1=-1.0,
                                    scalar2=None, op0=OP.mult)
            rstd = sm.tile([C, 1], f32, tag="rs" + stag)
            nc.scalar.activation(out=rstd, in_=vinv, func=AF.Sqrt)
            nmr = sm.tile([C, 1], f32, tag="nm" + stag)
            nc.scalar.activation(out=nmr, in_=rstd, func=AF.Identity,
                                 scale=nmu)
            return rstd, nmr

        def ln_finish(src_ps, rstd, nmr, gamma, betaresid, out_tile, stag,
                      eng):
            nrm = sm.tile([C, T], f32, tag="nr" + stag)
            nc.scalar.activation(out=nrm, in_=src_ps, func=AF.Identity,
                                 bias=nmr, scale=rstd)
            eng.tensor_tensor(out=out_tile, in0=nrm, in1=gamma, op=OP.mult)
            eng.tensor_tensor(out=out_tile, in0=out_tile, in1=betaresid,
                              op=OP.add)

        OUT_L = wk.tile([C, T], f32, tag="OUTL")
        OUT_R = wk.tile([C, T], f32, tag="OUTR")
        rl, nl = ln_start(ppl, "1l")
        rr, nr_ = ln_start(ppr, "1r")
        ln_finish(ppl, rl, nl, aglr, xlb, OUT_L, "1l", nc.vector)
        ln_finish(ppr, rr, nr_, agrr, xrb, OUT_R, "1r", nc.vector)
        dmy(attnT, 3)

        # ---- transposes -> [u(128), kt, c] f32 (router) + f16 oAll (experts) ----
        oLT2 = wk.tile([128, KT, C], f32, tag="oLT2")
        oRT2 = wk.tile([128, KT, C], f32, tag="oRT2")
        oAll = wk.tile([128, KT, 2, C], f16, tag="oAll")
        for (side, src, dst) in [(0, OUT_L, oLT2), (1, OUT_R, oRT2)]:
            for ut in range(KT):
                pt = ps.tile([128, C], f32, tag="ps")
                nc.tensor.transpose(pt, src[:, ut * 128:(ut + 1) * 128], ident)
                nc.vector.tensor_copy(dst[:, ut], pt)
                nc.scalar.copy(oAll[:, ut, side], pt)
            dmy(OUT_L, 1)

        # ---- router: xp.T then sims (no norms: top-k is scale-invariant
        #      per row and the top-2 softmax gate is exactly 1.0) ----
        rtiles = [(oLT2, 0), (oLT2, 1), (oRT2, 0), (oRT2, 1)]
        pxp = ps.tile([EXP, C], f32, tag="ps")
        for j, (tl, kt) in enumerate(rtiles):
            nc.tensor.matmul(pxp, wrt[:, j], tl[:, kt],
                             start=(j == 0), stop=(j == 3))
        xpT = sm.tile([EXP, C], f32, tag="xpT")
        nc.vector.tensor_scalar(out=xpT, in0=pxp, scalar1=brp, scalar2=None,
                                op0=OP.add)
        psim = ps.tile([C, C], f32, tag="ps")
        nc.tensor.matmul(psim, xpT, cent, start=True, stop=True)
        dmy(xpT, 4)  # dense dep-pinned burst: re-ramp HAM before experts

        # top-8 straight off PSUM
        mx8 = sm.tile([C, 8], f32, tag="mx8")
        nc.vector.max(out=mx8, in_=psim)
        idx8 = sm.tile([C, 8], mybir.dt.uint32, tag="idx8")
        nc.vector.max_index(out=idx8, in_max=mx8, in_values=psim)
        topif = sm.tile([C, 2], f32, tag="topif")
        nc.vector.tensor_copy(topif, idx8[:, 0:2])
        dmy(xpT, 2)

        # ---- replicate topi rows across partitions via PE ----
        ptt = ps.tile([2, C], f32, tag="ps")
        nc.tensor.transpose(ptt, topif, ident)
        ttT = sm.tile([2, C], f32, tag="ttT")
        nc.vector.tensor_copy(ttT, ptt)
        dmy(xpT, 2)
        pr0 = ps.tile([128, C], f32, tag="ps")
        nc.tensor.matmul(pr0, sel[:, 0], ttT, start=True, stop=True)
        pr1 = ps.tile([128, C], f32, tag="ps")
        nc.tensor.matmul(pr1, sel[:, 1], ttT, start=True, stop=True)
        tt0r = wk.tile([128, C], f16, tag="tt0r")
        tt1r = wk.tile([128, C], f16, tag="tt1r")
        nc.vector.tensor_copy(tt0r, pr0)
        nc.scalar.copy(tt1r, pr1)
        dmy(xpT, 4)

        # ---- R.T[e, c] (f16) on gpsimd (off the DVE critical path) ----
        RT0 = sm.tile([C, C], f16, tag="RT0")
        RT1 = sm.tile([C, C], f16, tag="RT1")
        nc.gpsimd.tensor_scalar(out=RT0, in0=tt0r[0:C], scalar1=eiota,
                                scalar2=None, op0=OP.is_equal)
        nc.gpsimd.tensor_scalar(out=RT1, in0=tt1r[0:C], scalar1=eiota,
                                scalar2=None, op0=OP.is_equal)
        RThh = wk.tile([C, C], f16, tag="RThh")
        nc.gpsimd.tensor_add(RThh, RT0, RT1)

        # experts 16..63 masks: DRAM round-trip on the (empty) Activation
        # HWDGE queue -- one write + three pipelined broadcast-reads.
        rtd = dram.tile([C, C], f16)
        nc.sync.dma_start(out=rtd[:], in_=RThh)
        rsrc = rtd[:]
        mrep1s = []
        for rc in range(3):
            mt = wk.tile([128, 16, C], f16, tag=f"mrep1{rc}")
            src_ap = bass.AP(tensor=rsrc.tensor,
                             offset=rsrc.offset + (16 + 16 * rc) * C,
                             ap=[[0, 128], [C, 16], [1, C]])
            nc.sync.dma_start(out=mt, in_=src_ap)
            mrep1s.append(mt)

        # ---- expert stage (bias matmuls moved to the end; e0 starts
        #      the PSUM accumulation group) ----
        ps_moe = moe_p.tile([128, T], f32, tag="psmoe")
        EG = 4
        mrep0 = wk.tile([128, 16, C], f16, tag="mrep0")

        def asch_mult(dst, msrc_ap):
            # dst[p, e(EG), kt, side, c] = oAll[p, kt, side, c] * m[e, c]
            out_ap = bass.AP(tensor=dst.tensor, offset=dst.offset,
                             ap=[list(dst.ap[0]), [KT * 2 * C, EG],
                                 [1, KT * 2 * C]])
            in0 = bass.AP(tensor=oAll.tensor, offset=oAll.offset,
                          ap=[list(oAll.ap[0]), [0, EG], [1, KT * 2 * C]])
            nc.vector.tensor_tensor(out=out_ap, in0=in0, in1=msrc_ap, op=OP.mult)

        def mask_bcast_ap(mt, col0):
            # [p, e(EG), kt*side(bcast), c] view of a [128, ncols, C] tile
            return bass.AP(tensor=mt.tensor, offset=mt.offset + col0 * C,
                           ap=[list(mt.ap[0]), [C, EG], [0, KT * 2], [1, C]])

        for g in range(C // EG):
            e0 = g * EG
            if e0 < 16:
                # inline masks for experts 0..15 (cover the round-trip)
                m0 = msk_p.tile([128, EG, C], f16, tag="m0")
                in0a = bass.AP(tensor=tt0r.tensor, offset=tt0r.offset,
                               ap=[list(tt0r.ap[0]), [0, EG], [1, C]])
                in1a = bass.AP(tensor=e8h.tensor, offset=e8h.offset + e0,
                               ap=[list(e8h.ap[0]), [1, EG], [0, C]])
                nc.vector.tensor_tensor(out=m0, in0=in0a, in1=in1a,
                                        op=OP.is_equal)
                m1 = msk_p.tile([128, EG, C], f16, tag="m1")
                in0b = bass.AP(tensor=tt1r.tensor, offset=tt1r.offset,
                               ap=[list(tt1r.ap[0]), [0, EG], [1, C]])
                nc.vector.tensor_tensor(out=m1, in0=in0b, in1=in1a,
                                        op=OP.is_equal)
                nc.vector.tensor_add(mrep0[:, e0:e0 + EG], m0, m1)
                msrc = mask_bcast_ap(mrep0, e0)
            else:
                msrc = mask_bcast_ap(mrep1s[(e0 - 16) // 16], (e0 - 16) % 16)
            asch = asc_p.tile([128, EG, KT, 2, C], f16, tag="asc")
            asch_mult(asch, msrc)
            for i in range(EG):
                for kt in range(KT):
                    nc.tensor.matmul(
                        ps_moe, asch[:, i, kt], weq[:, e0 + i, kt],
                        start=(g == 0 and i == 0 and kt == 0),
                        stop=False,
                        skip_group_check=True)
            if g < 6:
                dmy(tt0r, 1, 64)

        # bias (be * WE_SCALE) via R matmuls, closing the PSUM group
        nc.tensor.matmul(ps_moe[0:C], RThh, behh, start=False, stop=False,
                         skip_group_check=True)
        nc.tensor.matmul(ps_moe[C:128], RThh, behh, start=False, stop=True,
                         skip_group_check=True)

        # ---- beta+residual for LN2 on gpsimd during the expert phase ----
        obl = wk.tile([C, T], f32, tag="obl")
        obr = wk.tile([C, T], f32, tag="obr")
        nc.gpsimd.tensor_add(obl, OUT_L, oblb)
        nc.gpsimd.tensor_add(obr, OUT_R, obrb)

        # ---- final LN + residual, split DMA out on the Activation queue ----
        ol2 = wk.tile([C, T], f32, tag="ol2")
        or2 = wk.tile([C, T], f32, tag="or2")
        r2l, n2l = ln_start(ps_moe[0:C], "2l")
        r2r, n2r = ln_start(ps_moe[C:128], "2r")
        ln_finish(ps_moe[0:C], r2l, n2l, mglr, obl, ol2, "2l", nc.vector)
        nc.scalar.dma_start(out=ol2_d.ap(), in_=ol2)
        ln_finish(ps_moe[C:128], r2r, n2r, mgrr, obr, or2, "2r", nc.vector)
        nc.scalar.dma_start(out=or2_d.ap(), in_=or2)

    nc.compile()
    return nc


def _tile_t(w):
    # (T_in, N) -> [128, T_in//128, N] partition-tiled
    t_in, n = w.shape
    return np.ascontiguousarray(w.reshape(t_in // 128, 128, n).transpose(1, 0, 2))


def _prep_in_maps(inputs):
    import ml_dtypes

    f = np.float32
    x_l, x_r = inputs["x_l"], inputs["x_r"]

    def rep(v):
        return np.repeat(np.asarray(v, f).reshape(1, T), C, axis=0)

    cen = np.asarray(inputs["centers"], f)
    cenn = cen / np.maximum(np.linalg.norm(cen, axis=-1, keepdims=True), 1e-12)
    sel = np.zeros((2, 2, 128), f)
    sel[0, 0, :] = 1.0
    sel[1, 1, :] = 1.0
    arrs = {
        "wqt": _tile_t(np.asarray(inputs["Wq"], f).T),
        "wkt": _tile_t(np.asarray(inputs["Wk"], f).T),
        "wvt": _tile_t(np.asarray(inputs["Wv"], f).T),
        "wpt": _tile_t(np.asarray(inputs["Wp"], f).T),
        "bqp": np.asarray(inputs["bq"], f).reshape(KT, 128).T,
        "bkp": np.asarray(inputs["bk"], f).reshape(KT, 128).T,
        "wrt": _tile_t(np.asarray(inputs["Wr"], f).T),
        "brp": np.asarray(inputs["br"], f).reshape(EXP, 1),
        "cent": np.ascontiguousarray(cenn.T),
        "ident": np.eye(64, dtype=f),
        "eiota": np.arange(C, dtype=f).reshape(C, 1),
        "e8": np.tile(np.arange(C, dtype=f), (128, 1)),
        "sel": sel,
        "bvr": rep(inputs["bv"]), "bpr": rep(inputs["bp"]),
        "aglr": rep(inputs["ag_l"]), "agrr": rep(inputs["ag_r"]),
        "mglr": rep(inputs["mg_l"]), "mgrr": rep(inputs["mg_r"]),
        "oblb": rep(inputs["mb_l"]), "obrb": rep(inputs["mb_r"]),
        "behs": np.asarray(inputs["be"], f) * WE_SCALE,
        "xlb": np.zeros((C, T), f), "xrb": np.zeros((C, T), f),
    }
    # We -> [128(t%128), C, KT, T(u)] fp8e4, x128 (layer_norm absorbs it)
    We = np.asarray(inputs["We"], f)
    WeT = We.transpose(0, 2, 1).reshape(C, KT, 128, T).transpose(2, 0, 1, 3)
    weq = np.ascontiguousarray(WeT * WE_SCALE).astype(ml_dtypes.float8_e4m3)

    def pack(spec, ncols, extra):
        blob = np.zeros((128, ncols), f)
        for name, parts, shape in spec:
            off, _, _ = BLOB_OFF[name]
            cols = int(np.prod(shape[1:]))
            a = extra[name] if name in extra else arrs[name]
            blob[0:parts, off:off + cols] = np.asarray(a, f).reshape(parts, cols)
        return blob

    blobB2 = pack(BLOB_B2_SPEC, NB2, {})
    abl = np.asarray(inputs["ab_l"], f).reshape(1, T)
    abr = np.asarray(inputs["ab_r"], f).reshape(1, T)
    in_maps = []
    for b in range(N_CORES):
        xtl = _tile_t(np.ascontiguousarray(np.asarray(x_l[b], f).T))
        xtr = _tile_t(np.ascontiguousarray(np.asarray(x_r[b], f).T))
        blobA = pack(BLOB_A_SPEC, NA, {"xtl": xtl, "xtr": xtr})
        blobB1 = pack(BLOB_B1_SPEC, NB1,
                      {"xlb": np.asarray(x_l[b], f) + abl,
                       "xrb": np.asarray(x_r[b], f) + abr})
        in_maps.append({"blobA": blobA, "blobB1": blobB1, "blobB2": blobB2,
                        "weq": weq})
    return in_maps


def kernel(**inputs) -> np.ndarray:
    from concourse.bass_utils import run_bass_kernel_spmd

    if "nc" not in _CACHE:
        _CACHE["nc"] = _build()
    nc = _CACHE["nc"]
    in_maps = _prep_in_maps(inputs)
    res = run_bass_kernel_spmd(nc, in_maps, list(range(N_CORES)))
    _CACHE["exec_time_ns"] = res.exec_time_ns
    out_l2 = np.stack([res.results[b]["ol2"] for b in range(N_CORES)])
    out_r2 = np.stack([res.results[b]["or2"] for b in range(N_CORES)])
    return np.stack([out_l2, out_r2]).astype(np.float32)
